# revision 1
# baseline (speedup 1.0000x reference)
"""DAHead (dual-attention head) Trainium2 kernel.

8-core SPMD: core c handles sample c//2, spatial half c%2.
The half-split uses a flip trick so every core runs the SAME program:
odd cores receive the sample vertically flipped (and conv weights
flipped along dy); conv/attention/upsample all commute with the flip,
and the host un-flips the output half.

Per-core program:
  1. conv3x3+BN+lrelu (PAM) in bf16x2 (3-term hi/lo split - near-fp32,
     needed because the softmax downstream is argmax-like), features
     kept as bf16 hi/lo pairs.
  2. q/k (fp32 evac of bf16x2 matmuls) and v^T (bf16x2) projections,
     spilled to DRAM scratch (keeps SBUF pool lifetimes nested).
  3. Attention over own i-range (local rows 0..33): logits in plain
     fp32, softmax (ACT exp + DVE), PE transpose of the prob rows,
     apply in fp32r, residual, 1x1 out-proj in bf16x2.
  4. conv3x3+BN+lrelu (CAM) in fp32r, channel attention (mean -> MLP ->
     sigmoid, scale folded into the 1x1 weights), 1x1 out-proj fp32r,
     accumulated into the PAM output.
  5. bilinear x2 upsample (DVE) of the own half, DMA out.
"""
import sys

if '/opt/trn_rl_repo' not in sys.path:
    sys.path.insert(0, '/opt/trn_rl_repo')

import numpy as np
import ml_dtypes

import concourse.bass as bass
import concourse.mybir as mybir
import concourse.tile as tile
from concourse import bacc
from concourse.bass_utils import run_bass_kernel_spmd

dt = mybir.dt
f32 = dt.float32
f32r = dt.float32r
bf16 = dt.bfloat16
fp16 = dt.float16
BF = ml_dtypes.bfloat16
AF = mybir.ActivationFunctionType
OP = mybir.AluOpType

C = 512          # channels
P = 128          # partition size
NCH = C // P     # channel chunks (4)
H = W = 64
HW = H * W       # 4096
CR = 64          # q/k channels
OC = 64          # output channels
OWN_ROWS = 34    # local rows handled per core (rows 0..33)
OWN = OWN_ROWS * W    # 2176 = 17*128
NIC = OWN // P        # 17 attention i-chunks
NJC = HW // P         # 32 j-chunks
HB = 8                # conv h-blocks of 8 rows
EPS = 1e-5

# tap order: full-coverage center tap first (needed for PSUM start flag)
_ALL = [(ci, dy, dx) for ci in range(NCH) for dy in (-1, 0, 1) for dx in (-1, 0, 1)]
TAPS = [(0, 0, 0)] + [t for t in _ALL if t != (0, 0, 0)]
NT = len(TAPS)   # 36

# block edges for reads of the (OWN | rest) split f store
K_EDGES = [0, 512, 1024, 1536, 2048, OWN, OWN + 512, OWN + 1024, OWN + 1424, HW]
Q_EDGES = [0, 512, 1024, 1536, 2048, OWN]


def _conv_tap_aps(psum_t, x_t, hb, dy, dx):
    """APs for one conv tap on h-block hb. psum_t: [128,8,64], x_t: [128,64,64]."""
    r0 = 1 if (hb == 0 and dy == -1) else 0
    r1 = 7 if (hb == HB - 1 and dy == 1) else 8
    c0 = 1 if dx == -1 else 0
    c1 = 63 if dx == 1 else 64
    out_ap = psum_t[:, r0:r1, c0:c1]
    in_ap = x_t[:, hb * 8 + r0 + dy: hb * 8 + r1 + dy, c0 + dx: c1 + dx]
    return out_ap, in_ap


def _emit_phase1(nc, tc, d, ct, f_store):
    """PAM conv3x3 + BN + lrelu in bf16x2."""
    with tc.tile_pool(name="xs_pam", bufs=1) as px, \
         tc.tile_pool(name="wpam", bufs=3) as pw, \
         tc.tile_pool(name="pam_evac", bufs=3) as pe, \
         tc.tile_pool(name="ps_conv", bufs=1, space="PSUM") as psc:
        xh_t = [px.tile([P, H, W], bf16, name=f"xh{i}", tag=f"xh{i}") for i in range(NCH)]
        xl_t = [px.tile([P, H, W], bf16, name=f"xl{i}", tag=f"xl{i}") for i in range(NCH)]
        for i in range(NCH):
            nc.sync.dma_start(out=xh_t[i], in_=d['xh'][i])
            nc.sync.dma_start(out=xl_t[i], in_=d['xl'][i])
        for co in range(NCH):
            wh = pw.tile([P, NT, P], bf16, tag="w", name="wh")
            wl = pw.tile([P, NT, P], bf16, tag="w", name="wl")
            nc.sync.dma_start(out=wh, in_=d['wph'][co].rearrange("t p f -> p t f"))
            nc.sync.dma_start(out=wl, in_=d['wpl'][co].rearrange("t p f -> p t f"))
            pst = [psc.tile([P, 8, W], f32, tag=f"cv{hb}", name=f"cv{hb}")
                   for hb in range(HB)]
            for t, (ci, dy, dx) in enumerate(TAPS):
                for term in range(3):
                    wt = wh if term < 2 else wl
                    xt = xh_t if term != 1 else xl_t
                    for hb in range(HB):
                        o_ap, i_ap = _conv_tap_aps(pst[hb], xt[ci], hb, dy, dx)
                        nc.tensor.matmul(
                            o_ap, wt[:, t, :], i_ap,
                            start=(t == 0 and term == 0),
                            stop=(t == NT - 1 and term == 2))
            for hb in range(HB):
                z = pe.tile([P, 8 * W], f32, tag="z", name="z")
                nc.scalar.activation(
                    out=z, in_=pst[hb].rearrange("p a b -> p (a b)"),
                    func=AF.Identity, bias=ct['bp'][co], scale=ct['sp'][co])
                ft = pe.tile([P, 8 * W], f32, tag="ft", name="ft")
                nc.vector.scalar_tensor_tensor(
                    out=ft, in0=z, scalar=0.2, in1=z, op0=OP.mult, op1=OP.max)
                f_store(co, hb * 8 * W, ft)


def _emit_phase2(nc, tc, d, ct, f_read, qsc, ksc, vsc):
    """q/k (bf16x2, fp32 result) and vT (bf16x2) projections -> DRAM scratch."""
    with tc.tile_pool(name="qk_sb", bufs=1) as pqs, \
         tc.tile_pool(name="qk_w", bufs=1) as pqw, \
         tc.tile_pool(name="v_ev", bufs=2) as pve, \
         tc.tile_pool(name="ps_qkv", bufs=2, space="PSUM") as psq:
        q_t = pqs.tile([CR, OWN], f32, name="q_t", tag="q_t")
        k_t = pqs.tile([CR, HW], f32, name="k_t", tag="k_t")
        wq_h = [pqw.tile([P, CR], bf16, name=f"wqh{i}", tag=f"wqh{i}") for i in range(NCH)]
        wq_l = [pqw.tile([P, CR], bf16, name=f"wql{i}", tag=f"wql{i}") for i in range(NCH)]
        wk_h = [pqw.tile([P, CR], bf16, name=f"wkh{i}", tag=f"wkh{i}") for i in range(NCH)]
        wk_l = [pqw.tile([P, CR], bf16, name=f"wkl{i}", tag=f"wkl{i}") for i in range(NCH)]
        wv_h = [pqw.tile([P, C], bf16, name=f"wvh{i}", tag=f"wvh{i}") for i in range(NCH)]
        wv_l = [pqw.tile([P, C], bf16, name=f"wvl{i}", tag=f"wvl{i}") for i in range(NCH)]
        bv_t = pqw.tile([P, C], f32, name="bv_t", tag="bv_t")
        nc.sync.dma_start(out=bv_t, in_=d['bv'].to_broadcast([P, C]))
        for i in range(NCH):
            nc.sync.dma_start(out=wq_h[i], in_=d['wqh'][i])
            nc.sync.dma_start(out=wq_l[i], in_=d['wql'][i])
            nc.sync.dma_start(out=wk_h[i], in_=d['wkh'][i])
            nc.sync.dma_start(out=wk_l[i], in_=d['wkl'][i])
            nc.sync.dma_start(out=wv_h[i], in_=d['wvh'][i])
            nc.sync.dma_start(out=wv_l[i], in_=d['wvl'][i])

        def proj_qk(dst, wts_h, wts_l, bias_t, edges):
            for bi in range(len(edges) - 1):
                off, end = edges[bi], edges[bi + 1]
                sz = end - off
                pq = psq.tile([CR, 512], f32, tag="pq", name="pq")[:, 0:sz]
                first = True
                for ci in range(NCH):
                    # terms: (w_hi,f_hi), (w_hi,f_lo), (w_lo,f_hi)
                    for term in range(3):
                        wt = wts_h[ci] if term < 2 else wts_l[ci]
                        xin = f_read(term != 1, ci, off, end)
                        nc.tensor.matmul(pq, wt, xin, start=first,
                                         stop=(ci == NCH - 1 and term == 2))
                        first = False
                nc.scalar.activation(out=dst[:, off:end], in_=pq,
                                     func=AF.Identity, bias=bias_t, scale=1.0)

        proj_qk(q_t, wq_h, wq_l, ct['bq'], Q_EDGES)
        proj_qk(k_t, wk_h, wk_l, ct['bk'], K_EDGES)
        nc.sync.dma_start(out=qsc, in_=q_t)
        nc.sync.dma_start(out=ksc, in_=k_t)

        for jc in range(NJC):
            pv = psq.tile([P, C], f32, tag="pv", name="pv")
            s, e = jc * P, (jc + 1) * P
            # terms: (f_hi,wv_hi), (f_hi,wv_lo), (f_lo,wv_hi)
            for term in range(3):
                for ci in range(NCH):
                    lhs = f_read(term != 2, ci, s, e)
                    rhs = (wv_l if term == 1 else wv_h)[ci]
                    nc.tensor.matmul(pv, lhs, rhs,
                                     start=(term == 0 and ci == 0),
                                     stop=(term == 2 and ci == NCH - 1))
            vtmp = pve.tile([P, C], fp16, tag="vtmp", name="vtmp")
            nc.vector.tensor_add(vtmp, pv, bv_t)
            nc.sync.dma_start(out=vsc[:, jc, :], in_=vtmp)


def _emit_attention(nc, tc, ct, pam_sb, fsc, qsc, ksc, vsc):
    with tc.tile_pool(name="qk2", bufs=1) as pq2, \
         tc.tile_pool(name="vt2", bufs=1) as pv2, \
         tc.tile_pool(name="ls", bufs=1) as pls, \
         tc.tile_pool(name="et", bufs=1) as pet, \
         tc.tile_pool(name="fstream", bufs=2) as pfs, \
         tc.tile_pool(name="att_tmp", bufs=2) as pat, \
         tc.tile_pool(name="res_t", bufs=2) as prs, \
         tc.tile_pool(name="ps_l", bufs=2, space="PSUM") as psl, \
         tc.tile_pool(name="ps_t", bufs=2, space="PSUM") as pstp, \
         tc.tile_pool(name="ps_a", bufs=2, space="PSUM") as psa, \
         tc.tile_pool(name="ps_p", bufs=1, space="PSUM") as psp:
        q_t = pq2.tile([CR, OWN], f32, name="q2_t", tag="q2_t")
        k_t = pq2.tile([CR, HW], f32, name="k2_t", tag="k2_t")
        vt_t = pv2.tile([P, NJC, C], fp16, name="vt2_t", tag="vt2_t")
        nc.sync.dma_start(out=q_t, in_=qsc)
        nc.sync.dma_start(out=k_t, in_=ksc)
        nc.sync.dma_start(out=vt_t, in_=vsc)
        n_blocks = (NIC + 1) // 2
        for ib in range(n_blocks):
            ics = [2 * ib, 2 * ib + 1]
            if ics[-1] >= NIC:
                ics = ics[:1]
            isz = P * len(ics)
            ioff = ics[0] * P
            et_t = pet.tile([P, NJC, 2 * P], fp16, tag="et", name="et")
            for ph, ic in enumerate(ics):
                ls = pls.tile([P, HW], f32, tag="ls", name="ls")
                for jb in range(HW // 512):
                    pl = psl.tile([P, 512], f32, tag="pl", name="pl")
                    nc.tensor.matmul(
                        pl, q_t[:, ic * P:(ic + 1) * P],
                        k_t[:, jb * 512:(jb + 1) * 512], start=True, stop=True)
                    nc.scalar.activation(
                        out=ls[:, jb * 512:(jb + 1) * 512], in_=pl,
                        func=AF.Identity, bias=0.0, scale=1.0)
                nmax = pat.tile([P, 1], f32, tag="nmax", name="nmax")
                nc.vector.tensor_reduce(out=nmax, in_=ls, axis=mybir.AxisListType.X,
                                        op=OP.max, negate=True)
                rsum = pat.tile([P, 1], f32, tag="rsum", name="rsum")
                nc.scalar.activation(out=ls, in_=ls, func=AF.Exp,
                                     bias=nmax, scale=1.0, accum_out=rsum)
                rrec = pat.tile([P, 1], f32, tag="rrec", name="rrec")
                nc.vector.reciprocal(out=rrec, in_=rsum)
                e16 = pls.tile([P, HW], fp16, tag="e16", name="e16")
                nc.vector.tensor_scalar_mul(e16, ls, rrec)
                for jc in range(NJC):
                    pt = pstp.tile([P, P], fp16, tag="pt", name="pt")
                    nc.tensor.transpose(
                        pt, e16[:, jc * P:(jc + 1) * P], ct['ident'])
                    nc.vector.tensor_copy(
                        out=et_t[:, jc, ph * P:(ph + 1) * P], in_=pt)
            res_h, res_l = [], []
            for co in range(NCH):
                fs_h = pfs.tile([P, 2 * P], bf16, tag=f"fsh{co}",
                                name=f"fsh{co}")[:, 0:isz]
                fs_l = pfs.tile([P, 2 * P], bf16, tag=f"fsl{co}",
                                name=f"fsl{co}")[:, 0:isz]
                nc.sync.dma_start(out=fs_h, in_=fsc[co, 0, :, ioff:ioff + isz])
                nc.sync.dma_start(out=fs_l, in_=fsc[co, 1, :, ioff:ioff + isz])
                pa = psa.tile([P, 2 * P], f32, tag="pa", name="pa")[:, 0:isz]
                for jc in range(NJC):
                    nc.tensor.matmul(
                        pa, vt_t[:, jc, co * P:(co + 1) * P],
                        et_t[:, jc, 0:isz],
                        start=(jc == 0), stop=(jc == NJC - 1))
                rt = prs.tile([P, 2 * P], f32, tag="rt", name="rt")[:, 0:isz]
                nc.vector.scalar_tensor_tensor(
                    out=rt, in0=pa, scalar=ct['alpha'], in1=fs_h,
                    op0=OP.mult, op1=OP.add)
                nc.vector.tensor_add(rt, rt, fs_l)
                rh = prs.tile([P, 2 * P], bf16, tag=f"rh{co}", name=f"rh{co}")[:, 0:isz]
                nc.vector.tensor_copy(out=rh, in_=rt)
                rl = prs.tile([P, 2 * P], bf16, tag=f"rl{co}", name=f"rl{co}")[:, 0:isz]
                nc.vector.tensor_sub(rl, rt, rh)
                res_h.append(rh)
                res_l.append(rl)
            pp = psp.tile([OC, 2 * P], f32, tag="pp", name="pp")[:, 0:isz]
            first = True
            for ci in range(NCH):
                for term in range(3):
                    wt = ct['wpoh'][ci] if term < 2 else ct['wpol'][ci]
                    rs = res_h[ci] if term != 1 else res_l[ci]
                    nc.tensor.matmul(pp, wt, rs, start=first,
                                     stop=(ci == NCH - 1 and term == 2))
                    first = False
            nc.scalar.activation(out=pam_sb[:, ioff:ioff + isz], in_=pp,
                                 func=AF.Identity, bias=ct['bpo'], scale=1.0)


def _emit_cam(nc, tc, d, ct, pam_sb):
    with tc.tile_pool(name="xs_cam", bufs=1) as pxc, \
         tc.tile_pool(name="g_store", bufs=1) as pg:
        x_t = [pxc.tile([P, H, W], fp16, name=f"x{i}", tag=f"x{i}") for i in range(NCH)]
        for i in range(NCH):
            nc.sync.dma_start(out=x_t[i], in_=d['xs'][i])
        g_t = [pg.tile([P, HW], fp16, name=f"g{i}", tag=f"g{i}") for i in range(NCH)]
        with tc.tile_pool(name="wcam", bufs=2) as pwc, \
             tc.tile_pool(name="ps_conv2", bufs=1, space="PSUM") as psc2:
            for co in range(NCH):
                wc = pwc.tile([P, NT, P], fp16, tag="wc", name="wc")
                nc.sync.dma_start(out=wc, in_=d['wcm'][co].rearrange("t p f -> p t f"))
                pst = [psc2.tile([P, 8, W], f32, tag=f"cv{hb}", name=f"cv{hb}")
                       for hb in range(HB)]
                for t, (ci, dy, dx) in enumerate(TAPS):
                    for hb in range(HB):
                        o_ap, i_ap = _conv_tap_aps(pst[hb], x_t[ci], hb, dy, dx)
                        nc.tensor.matmul(
                            o_ap, wc[:, t, :], i_ap,
                            start=(t == 0), stop=(t == NT - 1))
                for hb in range(HB):
                    gsl = g_t[co][:, hb * 8 * W:(hb + 1) * 8 * W]
                    zc2 = pwc.tile([P, 8 * W], f32, tag="zc2", name="zc2")
                    nc.scalar.activation(
                        out=zc2, in_=pst[hb].rearrange("p a b -> p (a b)"),
                        func=AF.Identity, bias=ct['bc'][co], scale=ct['sc'][co])
                    nc.vector.scalar_tensor_tensor(
                        out=gsl, in0=zc2, scalar=0.2, in1=zc2,
                        op0=OP.mult, op1=OP.max)
        # channel attention MLP + 1x1 out
        with tc.tile_pool(name="mlp", bufs=1) as pm, \
             tc.tile_pool(name="cam_ev", bufs=2) as pce, \
             tc.tile_pool(name="ps_mlp", bufs=2, space="PSUM") as psm, \
             tc.tile_pool(name="ps_co", bufs=2, space="PSUM") as psco:
            msum = [pm.tile([P, 1], f32, name=f"ms{i}", tag=f"ms{i}") for i in range(NCH)]
            for i in range(NCH):
                nc.vector.tensor_reduce(out=msum[i], in_=g_t[i],
                                        axis=mybir.AxisListType.X, op=OP.add)
            wc1_t = [pm.tile([P, CR], f32, name=f"w1{i}", tag=f"w1{i}") for i in range(NCH)]
            wc2_t = [pm.tile([CR, P], f32, name=f"w2{i}", tag=f"w2{i}") for i in range(NCH)]
            wco_t = [pm.tile([P, OC], f32, name=f"wo{i}", tag=f"wo{i}") for i in range(NCH)]
            bc2_t = [pm.tile([P, 1], f32, name=f"b2{i}", tag=f"b2{i}") for i in range(NCH)]
            for i in range(NCH):
                nc.sync.dma_start(out=wc1_t[i], in_=d['wc1'][i])
                nc.sync.dma_start(out=wc2_t[i], in_=d['wc2'][i])
                nc.sync.dma_start(out=wco_t[i], in_=d['wco'][i])
                nc.sync.dma_start(out=bc2_t[i], in_=d['bc2'][i])
            p1 = psm.tile([CR, 1], f32, tag="p1", name="p1")
            for ci in range(NCH):
                nc.tensor.matmul(p1, wc1_t[ci], msum[ci],
                                 start=(ci == 0), stop=(ci == NCH - 1))
            t1 = pm.tile([CR, 1], f32, name="t1", tag="t1")
            nc.scalar.activation(out=t1, in_=p1, func=AF.Identity,
                                 bias=ct['bc1'], scale=1.0)
            y1 = pm.tile([CR, 1], f32, name="y1", tag="y1")
            nc.vector.scalar_tensor_tensor(out=y1, in0=t1, scalar=0.2, in1=t1,
                                           op0=OP.mult, op1=OP.max)
            s_t = [pm.tile([P, 1], f32, name=f"s{i}", tag=f"s{i}") for i in range(NCH)]
            wce = [pm.tile([P, OC], fp16, name=f"we{i}", tag=f"we{i}") for i in range(NCH)]
            for co in range(NCH):
                p2 = psm.tile([P, 1], f32, tag="p2", name="p2")
                nc.tensor.matmul(p2, wc2_t[co], y1,
                                 start=True, stop=True)
                nc.scalar.activation(out=s_t[co], in_=p2, func=AF.Sigmoid,
                                     bias=bc2_t[co], scale=1.0)
                nc.vector.tensor_scalar_mul(wce[co], wco_t[co], s_t[co])
            for bi in range(len(Q_EDGES) - 1):
                off, end = Q_EDGES[bi], Q_EDGES[bi + 1]
                sz = end - off
                pco = psco.tile([OC, 512], f32, tag="pco", name="pco")[:, 0:sz]
                for ci in range(NCH):
                    nc.tensor.matmul(pco, wce[ci], g_t[ci][:, off:end],
                                     start=(ci == 0), stop=(ci == NCH - 1))
                zc = pce.tile([OC, 512], f32, tag="zc", name="zc")[:, 0:sz]
                nc.scalar.activation(out=zc, in_=pco, func=AF.Identity,
                                     bias=ct['bco'], scale=1.0)
                # total = pam_out + cam_out, accumulated in place
                nc.vector.tensor_add(pam_sb[:, off:end], pam_sb[:, off:end], zc)


def _emit_upsample(nc, tc, pam_sb, y_d):
    with tc.tile_pool(name="up", bufs=1) as pu:
        su = pam_sb.rearrange("p (a b) -> p a b", b=W)  # [OC,34,64]
        a_t = pu.tile([OC, OWN_ROWS, W], f32, name="a_t", tag="a_t")
        b_t = pu.tile([OC, OWN_ROWS, W], f32, name="b_t", tag="b_t")
        nc.vector.tensor_scalar_mul(a_t.rearrange("p a b -> p (a b)"), pam_sb, 0.75)
        nc.vector.tensor_scalar_mul(b_t.rearrange("p a b -> p (a b)"), pam_sb, 0.25)
        sh = pu.tile([OC, OWN_ROWS, W, 2], f32, name="sh", tag="sh")
        nc.vector.tensor_copy(out=sh[:, :, 0, 0], in_=su[:, :, 0])
        nc.vector.tensor_add(sh[:, :, 1:W, 0], b_t[:, :, 0:W - 1], a_t[:, :, 1:W])
        nc.vector.tensor_add(sh[:, :, 0:W - 1, 1], a_t[:, :, 0:W - 1], b_t[:, :, 1:W])
        nc.vector.tensor_copy(out=sh[:, :, W - 1, 1], in_=su[:, :, W - 1])
        au = pu.tile([OC, OWN_ROWS, 2 * W], f32, name="au", tag="au")
        bu = pu.tile([OC, OWN_ROWS, 2 * W], f32, name="bu", tag="bu")
        shf = sh.rearrange("p a b c -> p a (b c)")
        nc.vector.tensor_scalar_mul(au.rearrange("p a b -> p (a b)"),
                                    shf.rearrange("p a b -> p (a b)"), 0.75)
        nc.vector.tensor_scalar_mul(bu.rearrange("p a b -> p (a b)"),
                                    shf.rearrange("p a b -> p (a b)"), 0.25)
        out_t = pu.tile([OC, H // 2, 2, 2 * W], f32, name="out_t", tag="out_t")
        nc.vector.tensor_copy(out=out_t[:, 0, 0, :], in_=shf[:, 0, :])
        nc.vector.tensor_add(out_t[:, 1:H // 2, 0, :], bu[:, 0:H // 2 - 1, :],
                             au[:, 1:H // 2, :])
        nc.vector.tensor_add(out_t[:, 0:H // 2, 1, :], au[:, 0:H // 2, :],
                             bu[:, 1:H // 2 + 1, :])
        nc.sync.dma_start(out=y_d, in_=out_t.rearrange("p a b c -> p (a b) c"))


def _build():
    nc = bacc.Bacc("TRN2", target_bir_lowering=False, debug=False,
                   enable_asserts=True, num_devices=8)

    def din(name, shape, dtp=f32):
        return nc.dram_tensor(name, shape, dtp, kind="ExternalInput").ap()

    d = {
        'xh': din("xh", [NCH, P, H, W], bf16),
        'xl': din("xl", [NCH, P, H, W], bf16),
        'xs': din("xs", [NCH, P, H, W], fp16),
        'wph': din("wph", [NCH, NT, P, P], bf16),
        'wpl': din("wpl", [NCH, NT, P, P], bf16),
        'wcm': din("wcm", [NCH, NT, P, P], fp16),
        'sp': din("sp", [NCH, P, 1]), 'bp': din("bp", [NCH, P, 1]),
        'sc': din("sc", [NCH, P, 1]), 'bc': din("bc", [NCH, P, 1]),
        'wqh': din("wqh", [NCH, P, CR], bf16), 'wql': din("wql", [NCH, P, CR], bf16),
        'wkh': din("wkh", [NCH, P, CR], bf16), 'wkl': din("wkl", [NCH, P, CR], bf16),
        'bq': din("bq", [CR, 1]), 'bk': din("bk", [CR, 1]),
        'wvh': din("wvh", [NCH, P, C], bf16), 'wvl': din("wvl", [NCH, P, C], bf16),
        'bv': din("bv", [1, C]),
        'alpha': din("alpha", [1, 1]),
        'wpoh': din("wpoh", [NCH, P, OC], bf16),
        'wpol': din("wpol", [NCH, P, OC], bf16),
        'bpo': din("bpo", [OC, 1]),
        'wc1': din("wc1", [NCH, P, CR]), 'bc1': din("bc1", [CR, 1]),
        'wc2': din("wc2", [NCH, CR, P]), 'bc2': din("bc2", [NCH, P, 1]),
        'wco': din("wco", [NCH, P, OC]), 'bco': din("bco", [OC, 1]),
        'ident': din("ident", [P, P], fp16),
    }
    y_d = nc.dram_tensor("y", [OC, H, 2 * W], f32, kind="ExternalOutput").ap()

    with tile.TileContext(nc) as tc:
        with tc.tile_pool(name="consts", bufs=2) as pc, \
             tc.tile_pool(name="fdram", bufs=1, space="DRAM") as pfd:
            ct = {}
            ct['ident'] = pc.tile([P, P], fp16, name="ident", tag="ident")
            nc.sync.dma_start(out=ct['ident'], in_=d['ident'])
            ct['alpha'] = pc.tile([P, 1], f32, name="alpha_t", tag="alpha_t")
            nc.sync.dma_start(out=ct['alpha'], in_=d['alpha'].to_broadcast([P, 1]))
            for nm, rows in (('bq', CR), ('bk', CR), ('bpo', OC), ('bco', OC),
                             ('bc1', CR)):
                ct[nm] = pc.tile([rows, 1], f32, name=f"{nm}_t", tag=f"{nm}_t")
                nc.sync.dma_start(out=ct[nm], in_=d[nm])
            for nm in ('sp', 'bp', 'sc', 'bc'):
                ct[nm] = [pc.tile([P, 1], f32, name=f"{nm}{i}_t", tag=f"{nm}{i}_t") for i in range(NCH)]
                for i in range(NCH):
                    nc.sync.dma_start(out=ct[nm][i], in_=d[nm][i])
            for nm in ('wpoh', 'wpol'):
                ct[nm] = [pc.tile([P, OC], bf16, name=f"{nm}{i}_t", tag=f"{nm}{i}_t")
                          for i in range(NCH)]
                for i in range(NCH):
                    nc.sync.dma_start(out=ct[nm][i], in_=d[nm][i])

            fsc = pfd.tile([NCH, 2, P, OWN], bf16, name="fsc", tag="fsc")
            qsc = pfd.tile([CR, OWN], f32, name="qsc", tag="qsc")
            ksc = pfd.tile([CR, HW], f32, name="ksc", tag="ksc")
            vsc = pfd.tile([P, NJC, C], fp16, name="vsc", tag="vsc")

            with tc.tile_pool(name="f_store", bufs=1) as p_f:
                fha = [p_f.tile([P, OWN], bf16, name=f"fha{i}", tag=f"fha{i}") for i in range(NCH)]
                fla = [p_f.tile([P, OWN], bf16, name=f"fla{i}", tag=f"fla{i}") for i in range(NCH)]
                fhb = [p_f.tile([P, HW - OWN], bf16, name=f"fhb{i}", tag=f"fhb{i}")
                       for i in range(NCH)]
                flb = [p_f.tile([P, HW - OWN], bf16, name=f"flb{i}", tag=f"flb{i}")
                       for i in range(NCH)]

                def f_store(co, off, src):
                    """split src ([128, n] f32 AP) into the bf16 hi/lo store."""
                    n = src.shape[-1]
                    pieces = []
                    if off < OWN:
                        k = min(OWN - off, n)
                        pieces.append((fha[co], fla[co], off, 0, k))
                    if off + n > OWN:
                        s_loc = max(OWN, off)
                        pieces.append((fhb[co], flb[co], s_loc - OWN, s_loc - off,
                                       off + n - s_loc))
                    for hi_t, lo_t, d0, s0, ln in pieces:
                        nc.vector.tensor_copy(out=hi_t[:, d0:d0 + ln],
                                              in_=src[:, s0:s0 + ln])
                        nc.vector.tensor_sub(lo_t[:, d0:d0 + ln],
                                             src[:, s0:s0 + ln],
                                             hi_t[:, d0:d0 + ln])

                def f_read(hi, co, s, e):
                    """AP for f[co][:, s:e]; must not cross the OWN boundary."""
                    if e <= OWN:
                        return (fha[co] if hi else fla[co])[:, s:e]
                    assert s >= OWN
                    return (fhb[co] if hi else flb[co])[:, s - OWN:e - OWN]

                _emit_phase1(nc, tc, d, ct, f_store)
                _emit_phase2(nc, tc, d, ct, f_read, qsc, ksc, vsc)
                # spill own-range features for residual streaming
                for co in range(NCH):
                    nc.sync.dma_start(out=fsc[co, 0], in_=fha[co])
                    nc.sync.dma_start(out=fsc[co, 1], in_=fla[co])

            with tc.tile_pool(name="pam_out", bufs=1) as p_pam:
                pam_sb = p_pam.tile([OC, OWN], f32, name="pam_sb", tag="pam_sb")
                _emit_attention(nc, tc, ct, pam_sb, fsc, qsc, ksc, vsc)
                _emit_cam(nc, tc, d, ct, pam_sb)
                _emit_upsample(nc, tc, pam_sb, y_d)
    nc.compile()
    return nc


_NC_CACHE = None


def _get_nc():
    global _NC_CACHE
    if _NC_CACHE is None:
        _NC_CACHE = _build()
    return _NC_CACHE


def _hi_lo(a):
    hi = np.asarray(a, np.float32).astype(BF)
    lo = (np.asarray(a, np.float32) - hi.astype(np.float32)).astype(BF)
    return hi, lo


_TAP_CI = np.array([t[0] for t in TAPS])
_TAP_DY = np.array([t[1] + 1 for t in TAPS])
_TAP_DX = np.array([t[2] + 1 for t in TAPS])


def _pack_conv(wfull):
    """[C, C, 3, 3] -> [NCH(co), NT, P(ci_local), P(co_local)] lhsT tiles."""
    wr = np.asarray(wfull, np.float32).reshape(NCH, P, NCH, P, 3, 3)
    wt = wr.transpose(0, 2, 4, 5, 3, 1)  # [co, ci, dy, dx, ci_l, co_l]
    return np.ascontiguousarray(wt[:, _TAP_CI, _TAP_DY, _TAP_DX])


def _packT(w, free):
    """w [free, C] -> [NCH, P, free] lhsT chunks."""
    return np.ascontiguousarray(np.asarray(w, np.float32).T.reshape(NCH, P, free))


def _prep_shared(inputs, flip):
    wp = np.asarray(inputs['W_pam_in'], np.float32)
    wc = np.asarray(inputs['W_cam_in'], np.float32)
    if flip:
        wp = wp[:, :, ::-1, :]
        wc = wc[:, :, ::-1, :]
    wph, wpl = _hi_lo(_pack_conv(wp))
    wcm = _pack_conv(wc).astype(np.float16)

    def bnfold(g, b, m, v):
        s = (np.asarray(g, np.float32)
             / np.sqrt(np.asarray(v, np.float32) + EPS)).astype(np.float32)
        bb = (np.asarray(b, np.float32)
              - np.asarray(m, np.float32) * s).astype(np.float32)
        return s.reshape(NCH, P, 1), bb.reshape(NCH, P, 1)

    sp, bp = bnfold(inputs['pam_gamma'], inputs['pam_beta'],
                    inputs['pam_mean'], inputs['pam_var'])
    sc, bc = bnfold(inputs['cam_gamma'], inputs['cam_beta'],
                    inputs['cam_mean'], inputs['cam_var'])
    wqh, wql = _hi_lo(_packT(inputs['Wq'], CR))
    wkh, wkl = _hi_lo(_packT(inputs['Wk'], CR))
    wvh, wvl = _hi_lo(_packT(inputs['Wv'], C))
    wpoh, wpol = _hi_lo(_packT(inputs['W_pam_out'], OC))
    # Wc2 [C, CR] -> lhsT chunks [NCH, CR, P]
    wc2 = np.ascontiguousarray(
        np.asarray(inputs['Wc2'], np.float32).reshape(NCH, P, CR).transpose(0, 2, 1))
    return {
        'wph': wph, 'wpl': wpl, 'wcm': wcm,
        'sp': sp, 'bp': bp, 'sc': sc, 'bc': bc,
        'wqh': wqh, 'wql': wql, 'wkh': wkh, 'wkl': wkl,
        'bq': np.asarray(inputs['bq'], np.float32).reshape(CR, 1),
        'bk': np.asarray(inputs['bk'], np.float32).reshape(CR, 1),
        'wvh': wvh, 'wvl': wvl,
        'bv': np.asarray(inputs['bv'], np.float32).reshape(1, C),
        'alpha': np.asarray(inputs['alpha'], np.float32).reshape(1, 1),
        'wpoh': wpoh, 'wpol': wpol,
        'bpo': np.asarray(inputs['b_pam_out'], np.float32).reshape(OC, 1),
        'wc1': _packT(np.asarray(inputs['Wc1'], np.float32) / HW, CR),
        'bc1': np.asarray(inputs['bc1'], np.float32).reshape(CR, 1),
        'wc2': wc2,
        'bc2': np.asarray(inputs['bc2'], np.float32).reshape(NCH, P, 1),
        'wco': _packT(inputs['W_cam_out'], OC),
        'ident': np.eye(P, dtype=np.float16),
        'bco': np.asarray(inputs['b_cam_out'], np.float32).reshape(OC, 1),
    }


def _make_in_maps(inputs):
    x = np.asarray(inputs['x'], np.float32)  # [4, 512, 64, 64]
    shared = {f: _prep_shared(inputs, f) for f in (False, True)}
    in_maps = []
    for c in range(8):
        s, flip = c // 2, c % 2
        xs = x[s]
        if flip:
            xs = xs[:, ::-1, :]
        xs = np.ascontiguousarray(xs.reshape(NCH, P, H, W))
        xhh, xll = _hi_lo(xs)
        m = dict(shared[bool(flip)])
        m['xs'] = xs.astype(np.float16)
        m['xh'] = xhh
        m['xl'] = xll
        in_maps.append(m)
    return in_maps


def kernel(**inputs):
    nc = _get_nc()
    in_maps = _make_in_maps(inputs)
    res = run_bass_kernel_spmd(nc, in_maps, list(range(8)))
    out = np.empty((4, OC, 2 * H, 2 * W), np.float32)
    for c in range(8):
        s, flip = c // 2, c % 2
        o = res.results[c]['y']  # [64, 64, 128]
        if flip:
            out[s, :, H:2 * H, :] = o[:, ::-1, :]
        else:
            out[s, :, 0:H, :] = o
    return out



# revision 58
# speedup vs baseline: 5.5791x; 5.5791x over previous
"""DAHead (dual-attention head) Trainium2 kernel, v2.

8-core SPMD: core c handles sample c//2, spatial half c%2 (odd cores get
the sample vertically flipped so every core runs the same program; conv
weights are dy-flipped to match and the host un-flips the output half).

v2 strategy vs v1: the two cores of a sample pair now SPLIT the heavy
convs spatially instead of both computing the full image.  Each core
convolves only its own 34 rows (+1 halo row of x), computes q for its
own 34 rows and k/v for its own 32-row token half, then the pair
exchanges k/v via AllGather (softmax and the weighted sum are
permutation-invariant over the token axis j, so the gathered halves can
stay in each core's local frame - no un-flipping needed).  The CAM
global mean is assembled with a tiny pair AllReduce of per-channel
partial sums.

Numerics: fp32r matmuls (~13 effective mantissa bits, full PE rate)
replace the v1 bf16x2 3-pass scheme for both convs, the v-projection
and the 1x1 output projections; q/k projections stay fp32 and the big
[i x j] logits matmul runs as a packed fp16 hi/lo 2-pass (error ~2^-21,
needed because the softmax is argmax-like).  End-to-end rel err vs the
fp64 reference is ~5e-3 (gate 2e-2).
"""
import sys

if '/opt/trn_rl_repo' not in sys.path:
    sys.path.insert(0, '/opt/trn_rl_repo')

import numpy as np
import ml_dtypes

import concourse.bass as bass
import concourse.mybir as mybir
import concourse.tile as tile
from concourse import bacc
from concourse.bass_utils import run_bass_kernel_spmd

dt = mybir.dt
f32 = dt.float32
f32r = dt.float32r
bf16 = dt.bfloat16
fp16 = dt.float16
BF = ml_dtypes.bfloat16
AF = mybir.ActivationFunctionType
OP = mybir.AluOpType

C = 512          # channels
P = 128          # partition size
NCH = C // P     # channel chunks (4)
H = W = 64
HW = H * W       # 4096
CR = 64          # q/k channels
OC = 64          # output channels
OWN_ROWS = 34    # rows convolved per core (local frame rows 0..33)
OWN = OWN_ROWS * W    # 2176 = 17*128
XR = OWN_ROWS + 1     # x rows loaded (halo row 34 feeds conv row 33)
# x is stored zero-padded: top pad row + left/right pad cols, so every conv
# tap is a full even-width window (fp32r matmul ISA requires even moving
# dims) and SAME padding falls out of the zero border.
XPR = XR + 1          # 36 rows (row 0 = zeros)
XPW = W + 2           # 66 cols (cols 0 and 65 = zeros)
NIC = OWN // P        # 17 attention i-chunks
JROWS = 32            # token rows owned per core (disjoint pair cover)
JOWN = JROWS * W      # 2048
NJC_OWN = JOWN // P   # 16
NJC = HW // P         # 32 j-chunks after the gather
EPS = 1e-5
GROUPS = [[0, 1], [2, 3], [4, 5], [6, 7]]

# conv h-blocks over the 34 own rows; every block >= 4 rows so the fp32r
# matmul moving dim stays >= 256 (full PE rate)
HBS = [(0, 7), (7, 7), (14, 7), (21, 7), (28, 6)]

# tap order: full-coverage center tap first (needed for PSUM start flag)
_ALL = [(ci, dy, dx) for ci in range(NCH) for dy in (-1, 0, 1) for dx in (-1, 0, 1)]
TAPS = [(0, 0, 0)] + [t for t in _ALL if t != (0, 0, 0)]
NT = len(TAPS)   # 36

Q_EDGES = [0, 512, 1024, 1536, 2048, OWN]
K_EDGES = [0, 512, 1024, 1536, JOWN]


def _conv_tap_aps(psum_t, x_t, row0, rows, dy, dx):
    """APs for one conv tap on the h-block [row0, row0+rows).

    x_t is zero-bordered [128, XPR, XPW] (data at [1:, 1:65]), so every
    tap reads a full rows x 64 window - no edge clamps, even moving dims.
    """
    out_ap = psum_t[:, 0:rows, :]
    in_ap = x_t[:, row0 + 1 + dy: row0 + 1 + dy + rows, 1 + dx: 1 + dx + W]
    return out_ap, in_ap


def _emit_conv(nc, tc, x_t, w_d, scale_t, bias_t, store, pools):
    """3x3 conv over own rows in fp32r + BN + lrelu; store(co, f32_ap).

    Weight DMAs ride the Activation queue so they are not stuck behind
    data DMAs (k/v spills) on the sync queue.  The tile pools are shared
    between the PAM and CAM instances so the second conv's weight
    prefetch reuses the first conv's space (clean WAR, no dependency on
    anything downstream of the collectives).
    """
    HT = NT // 2  # weights stream in tap-halves (smaller tiles, deeper prefetch)
    with tc.tile_pool(name="wconv", bufs=2) as pw, \
         tc.tile_pool(name="conv_evac", bufs=2) as pe, \
         tc.tile_pool(name="ps_conv", bufs=1, space="PSUM") as psc:
        for co in range(NCH):
            if co == 0 and pools is not None:
                wh = [pools[:, 0:HT, :], pools[:, HT:NT, :]]
            else:
                wh = []
                for h in range(2):
                    wt = pw.tile([P, HT, P], f32r, tag="w", name="w")
                    nc.scalar.dma_start(out=wt,
                                        in_=w_d[co][:, h * HT:(h + 1) * HT, :])
                    wh.append(wt)
            pst = [psc.tile([P, rows, W], f32, tag=f"cv{b}", name=f"cv{b}")
                   for b, (row0, rows) in enumerate(HBS)]
            for t, (ci, dy, dx) in enumerate(TAPS):
                for b, (row0, rows) in enumerate(HBS):
                    o_ap, i_ap = _conv_tap_aps(pst[b], x_t[ci], row0, rows,
                                               dy, dx)
                    nc.tensor.matmul(o_ap, wh[t // HT][:, t % HT, :], i_ap,
                                     start=(t == 0), stop=(t == NT - 1))
            for b, (row0, rows) in enumerate(HBS):
                z = pe.tile([P, 8 * W], f32, tag="z", name="z")[:, 0:rows * W]
                nc.scalar.activation(
                    out=z, in_=pst[b].rearrange("p a b -> p (a b)"),
                    func=AF.Identity, bias=bias_t[co], scale=scale_t[co])
                store(co, row0 * W, rows * W, z)


def _emit_qkv(nc, tc, d, ct, f_t, kin, vin):
    """q (own rows, fp32), k/v (own token half) -> DRAM, pair AllGather.

    k is hi/lo-split to fp16 BEFORE the gather, so the post-gather work
    is pure DMA (no DVE) and nothing downstream of the collective sits on
    an SBUF range the CAM conv wants to reuse.  q is packed right here
    too (local, no gather involved).
    """
    with tc.tile_pool(name="qk_w", bufs=1) as pqw, \
         tc.tile_pool(name="v_ev", bufs=2) as pve, \
         tc.tile_pool(name="ps_qkv", bufs=2, space="PSUM") as psq:
        wq_t = [pqw.tile([P, CR], f32, name=f"wq{i}", tag=f"wq{i}") for i in range(NCH)]
        wk_t = [pqw.tile([P, CR], f32, name=f"wk{i}", tag=f"wk{i}") for i in range(NCH)]
        wv_t = [pqw.tile([P, C], f32r, name=f"wv{i}", tag=f"wv{i}") for i in range(NCH)]
        bv_t = pqw.tile([P, C], f32, name="bv_t", tag="bv_t")
        nc.sync.dma_start(out=bv_t, in_=d['bv'].to_broadcast([P, C]))
        for i in range(NCH):
            nc.sync.dma_start(out=wq_t[i], in_=d['wq'][i])
            nc.sync.dma_start(out=wk_t[i], in_=d['wk'][i])
            nc.sync.dma_start(out=wv_t[i], in_=d['wv'][i])

        def proj(dst, wts, bias_t, edges):
            for bi in range(len(edges) - 1):
                off, end = edges[bi], edges[bi + 1]
                sz = end - off
                pq = psq.tile([CR, 512], f32, tag="pq", name="pq")[:, 0:sz]
                for ci in range(NCH):
                    nc.tensor.matmul(pq, wts[ci], f_t[ci][:, off:end].bitcast(f32),
                                     start=(ci == 0), stop=(ci == NCH - 1))
                nc.scalar.activation(out=dst[:, off:end], in_=pq,
                                     func=AF.Identity, bias=bias_t, scale=1.0)

        q32 = ct['q32']
        k32 = ct['k32']
        qpk, khd, klo = ct['qpk'], ct['khd'], ct['klo']

        proj(k32, wk_t, ct['bk'], K_EDGES)
        khl = pqw.tile([CR, 2, JOWN], fp16, name="khl", tag="khl")
        nc.vector.tensor_copy(out=khl[:, 0, :], in_=k32)
        nc.vector.tensor_sub(khl[:, 1, :], k32, khl[:, 0, :])
        nc.sync.dma_start(out=kin, in_=khl)
        nc.gpsimd.collective_compute(
            "AllGather", mybir.AluOpType.bypass, replica_groups=GROUPS,
            ins=[kin.opt()], outs=[ct['kout'].opt()])

        for jc in range(NJC_OWN):
            pv = psq.tile([P, C], f32, tag="pv", name="pv")
            s = jc * P
            for ci in range(NCH):
                nc.tensor.matmul(pv, f_t[ci][:, s:s + P], wv_t[ci],
                                 start=(ci == 0), stop=(ci == NCH - 1))
            vtmp = pve.tile([P, C], fp16, tag="vtmp", name="vtmp")
            nc.vector.tensor_add(vtmp, pv, bv_t)
            nc.sync.dma_start(out=vin[:, jc, :], in_=vtmp)
        nc.gpsimd.collective_compute(
            "AllGather", mybir.AluOpType.bypass, replica_groups=GROUPS,
            ins=[vin.opt()], outs=[ct['vout'].opt()])

        proj(q32, wq_t, ct['bq'], Q_EDGES)
        qlo = ct['qlo']
        nc.vector.tensor_copy(out=qpk[0:CR, :], in_=q32)
        nc.vector.tensor_sub(qlo, q32, qpk[0:CR, :])


def _emit_qk_unpack(nc, ct):
    """Post-gather k unpack + q lo move: pure DMAs, emitted after the CAM
    conv and on the ACT queue, so nothing that waits on the k gather sits
    ahead of the v spills or conv weight loads."""
    nc.scalar.dma_start(out=ct['khd'][0:CR, 0:JOWN], in_=ct['kout'][0, :, 0])
    nc.scalar.dma_start(out=ct['khd'][0:CR, JOWN:HW], in_=ct['kout'][1, :, 0])
    nc.scalar.dma_start(out=ct['klo'][:, 0:JOWN], in_=ct['kout'][0, :, 1])
    nc.scalar.dma_start(out=ct['klo'][:, JOWN:HW], in_=ct['kout'][1, :, 1])
    nc.scalar.dma_start(out=ct['khd'][CR:P, :], in_=ct['khd'][0:CR, :])
    nc.scalar.dma_start(out=ct['qpk'][CR:P, :], in_=ct['qlo'])


def _emit_attention(nc, tc, ct, f16_t, pam_sb, vt_t, ibs, post_cb=None):
    with tc.tile_pool(name="ls", bufs=2) as pls, \
         tc.tile_pool(name="et", bufs=2) as pet, \
         tc.tile_pool(name="att_tmp", bufs=2) as pat, \
         tc.tile_pool(name="res_t", bufs=2) as prs, \
         tc.tile_pool(name="ps_l", bufs=2, space="PSUM") as psl, \
         tc.tile_pool(name="ps_t", bufs=2, space="PSUM") as pstp, \
         tc.tile_pool(name="ps_a", bufs=2, space="PSUM") as psa, \
         tc.tile_pool(name="ps_p", bufs=1, space="PSUM") as psp:
        qpk, khd, klo = ct['qpk'], ct['khd'], ct['klo']
        for ib in ibs:
            ics = [2 * ib, 2 * ib + 1]
            if ics[-1] >= NIC:
                ics = ics[:1]
            isz = P * len(ics)
            ioff = ics[0] * P
            et_t = pet.tile([P, NJC, 2 * P], fp16, tag="et", name="et")
            for ph, ic in enumerate(ics):
                ls = pls.tile([P, HW], f32, tag="ls", name="ls")
                nmax8 = pat.tile([P, 8], f32, tag="nmax8", name="nmax8")
                for jb in range(HW // 512):
                    pl = psl.tile([P, 512], f32, tag="pl", name="pl")
                    nc.tensor.matmul(
                        pl, qpk[:, ic * P:(ic + 1) * P],
                        khd[:, jb * 512:(jb + 1) * 512], start=True, stop=False)
                    nc.tensor.matmul(
                        pl, qpk[0:CR, ic * P:(ic + 1) * P],
                        klo[:, jb * 512:(jb + 1) * 512], start=False, stop=True)
                    nc.scalar.activation(
                        out=ls[:, jb * 512:(jb + 1) * 512], in_=pl,
                        func=AF.Identity, bias=0.0, scale=1.0)
                    nc.vector.tensor_reduce(
                        out=nmax8[:, jb:jb + 1], in_=ls[:, jb * 512:(jb + 1) * 512],
                        axis=mybir.AxisListType.X, op=OP.max)
                nmax = pat.tile([P, 1], f32, tag="nmax", name="nmax")
                nc.vector.tensor_reduce(out=nmax, in_=nmax8,
                                        axis=mybir.AxisListType.X,
                                        op=OP.max, negate=True)
                rsum = pat.tile([P, 1], f32, tag="rsum", name="rsum")
                nc.scalar.activation(out=ls, in_=ls, func=AF.Exp,
                                     bias=nmax, scale=1.0, accum_out=rsum)
                rrec = pat.tile([P, 1], f32, tag="rrec", name="rrec")
                nc.vector.reciprocal(out=rrec, in_=rsum)
                e16 = pls.tile([P, HW], fp16, tag="e16", name="e16")
                nc.vector.tensor_scalar_mul(e16, ls, rrec)
                # transpose 128-col blocks; batch 4 per PSUM tile -> 1 evac
                for jq in range(NJC // 4):
                    pt = pstp.tile([P, 4 * P], fp16, tag="pt", name="pt")
                    for k in range(4):
                        jc = 4 * jq + k
                        nc.tensor.transpose(
                            pt[:, k * P:(k + 1) * P],
                            e16[:, jc * P:(jc + 1) * P], ct['ident'])
                    nc.vector.tensor_copy(
                        out=et_t[:, 4 * jq:4 * jq + 4, ph * P:(ph + 1) * P],
                        in_=pt.rearrange("p (a b) -> p a b", b=P))
            pp = psp.tile([OC, 2 * P], f32, tag="pp", name="pp")[:, 0:isz]
            for co in range(NCH):
                pa = psa.tile([P, 2 * P], f32, tag="pa", name="pa")[:, 0:isz]
                for jc in range(NJC):
                    nc.tensor.matmul(
                        pa, vt_t[:, jc, co * P:(co + 1) * P],
                        et_t[:, jc, 0:isz],
                        start=(jc == 0), stop=(jc == NJC - 1))
                rt = prs.tile([P, 2 * P], f32r, tag="rt", name="rt")[:, 0:isz]
                nc.vector.scalar_tensor_tensor(
                    out=rt, in0=pa, scalar=ct['alpha'],
                    in1=f16_t[co][:, ioff:ioff + isz], op0=OP.mult, op1=OP.add)
                nc.tensor.matmul(pp, ct['wpo'][co], rt,
                                 start=(co == 0), stop=(co == NCH - 1))
            nc.scalar.activation(out=pam_sb[:, ioff:ioff + isz], in_=pp,
                                 func=AF.Identity, bias=ct['bpo'], scale=1.0)
            if post_cb and ib in post_cb:
                post_cb[ib]()


def _emit_cam_mlp(nc, tc, d, ct, g_t, zc_full):
    """channel-attention MLP (on the pair-reduced mean) + 1x1 out-proj.

    Emitted between the two attention halves so its matmuls overlap the
    attention stream and its (tiny) wait on the mean AllReduce is hidden.
    Writes the CAM branch output into zc_full; the add into pam_sb
    happens after the second attention half.
    """
    with tc.tile_pool(name="mlp", bufs=1) as pm, \
         tc.tile_pool(name="ps_mlp", bufs=2, space="PSUM") as psm, \
         tc.tile_pool(name="ps_co", bufs=2, space="PSUM") as psco:
        msum = [pm.tile([P, 1], f32, name=f"ms{i}", tag=f"ms{i}") for i in range(NCH)]
        for i in range(NCH):
            nc.sync.dma_start(out=msum[i], in_=ct['mout'][i])
        wc1_t = [pm.tile([P, CR], f32, name=f"w1{i}", tag=f"w1{i}") for i in range(NCH)]
        wc2_t = [pm.tile([CR, P], f32, name=f"w2{i}", tag=f"w2{i}") for i in range(NCH)]
        wco_t = [pm.tile([P, OC], f32, name=f"wo{i}", tag=f"wo{i}") for i in range(NCH)]
        bc2_t = [pm.tile([P, 1], f32, name=f"b2{i}", tag=f"b2{i}") for i in range(NCH)]
        for i in range(NCH):
            nc.sync.dma_start(out=wc1_t[i], in_=d['wc1'][i])
            nc.sync.dma_start(out=wc2_t[i], in_=d['wc2'][i])
            nc.sync.dma_start(out=wco_t[i], in_=d['wco'][i])
            nc.sync.dma_start(out=bc2_t[i], in_=d['bc2'][i])
        p1 = psm.tile([CR, 1], f32, tag="p1", name="p1")
        for ci in range(NCH):
            nc.tensor.matmul(p1, wc1_t[ci], msum[ci],
                             start=(ci == 0), stop=(ci == NCH - 1))
        t1 = pm.tile([CR, 1], f32, name="t1", tag="t1")
        nc.scalar.activation(out=t1, in_=p1, func=AF.Identity,
                             bias=ct['bc1'], scale=1.0)
        y1 = pm.tile([CR, 1], f32, name="y1", tag="y1")
        nc.vector.scalar_tensor_tensor(out=y1, in0=t1, scalar=0.2, in1=t1,
                                       op0=OP.mult, op1=OP.max)
        wce = [pm.tile([P, OC], fp16, name=f"we{i}", tag=f"we{i}") for i in range(NCH)]
        for co in range(NCH):
            p2 = psm.tile([P, 1], f32, tag="p2", name="p2")
            nc.tensor.matmul(p2, wc2_t[co], y1, start=True, stop=True)
            s_t = pm.tile([P, 1], f32, name=f"s{co}", tag=f"s{co}")
            nc.scalar.activation(out=s_t, in_=p2, func=AF.Sigmoid,
                                 bias=bc2_t[co], scale=1.0)
            nc.vector.tensor_scalar_mul(wce[co], wco_t[co], s_t)
        for bi in range(len(Q_EDGES) - 1):
            off, end = Q_EDGES[bi], Q_EDGES[bi + 1]
            sz = end - off
            pco = psco.tile([OC, 512], f32, tag="pco", name="pco")[:, 0:sz]
            for ci in range(NCH):
                nc.tensor.matmul(pco, wce[ci], g_t[ci][:, off:end],
                                 start=(ci == 0), stop=(ci == NCH - 1))
            nc.scalar.activation(out=zc_full[:, off:end], in_=pco,
                                 func=AF.Identity, bias=ct['bco'], scale=1.0)


UPR = 4   # output row-pairs per upsample chunk
UPN = UPR + 2  # max su rows a chunk reads


def _emit_up_chunk(nc, pu, pam_sb, zc_full, y_d, r0, r1, addlo, addhi):
    """CAM add for pam rows [addlo,addhi) + bilinear x2 of su rows giving
    output rows [2*r0, 2*r1).  Needs su rows [r0-1, r1] complete (i.e. the
    attention writes AND the cam adds for those rows)."""
    ad = pam_sb[:, addlo * W:addhi * W]
    nc.vector.tensor_add(ad, ad, zc_full[:, addlo * W:addhi * W])
    su = pam_sb.rearrange("p (a b) -> p a b", b=W)  # [OC,34,64]
    in_lo = max(r0 - 1, 0)
    n = r1 - in_lo + 1          # su rows [in_lo, r1]
    m = r1 - r0
    base = r0 - in_lo
    a_t = pu.tile([OC, UPN, W], f32, name="a_t", tag="a_t")
    b_t = pu.tile([OC, UPN, W], f32, name="b_t", tag="b_t")
    seg = pam_sb[:, in_lo * W:(r1 + 1) * W]
    nc.vector.tensor_scalar_mul(
        a_t.rearrange("p a b -> p (a b)")[:, 0:n * W], seg, 0.75)
    nc.vector.tensor_scalar_mul(
        b_t.rearrange("p a b -> p (a b)")[:, 0:n * W], seg, 0.25)
    sh = pu.tile([OC, UPN, W, 2], f32, name="sh", tag="sh")
    nc.vector.tensor_copy(out=sh[:, 0:n, 0, 0], in_=su[:, in_lo:r1 + 1, 0])
    nc.vector.tensor_add(sh[:, 0:n, 1:W, 0], b_t[:, 0:n, 0:W - 1],
                         a_t[:, 0:n, 1:W])
    nc.vector.tensor_add(sh[:, 0:n, 0:W - 1, 1], a_t[:, 0:n, 0:W - 1],
                         b_t[:, 0:n, 1:W])
    nc.vector.tensor_copy(out=sh[:, 0:n, W - 1, 1], in_=su[:, in_lo:r1 + 1, W - 1])
    au = pu.tile([OC, UPN, 2 * W], f32, name="au", tag="au")
    bu = pu.tile([OC, UPN, 2 * W], f32, name="bu", tag="bu")
    shf = sh.rearrange("p a b c -> p a (b c)")
    nc.vector.tensor_scalar_mul(
        au.rearrange("p a b -> p (a b)")[:, 0:n * 2 * W],
        shf.rearrange("p a b -> p (a b)")[:, 0:n * 2 * W], 0.75)
    nc.vector.tensor_scalar_mul(
        bu.rearrange("p a b -> p (a b)")[:, 0:n * 2 * W],
        shf.rearrange("p a b -> p (a b)")[:, 0:n * 2 * W], 0.25)
    out_t = pu.tile([OC, UPR, 2, 2 * W], f32, name="out_t", tag="out_t")
    j0 = 1 if r0 == 0 else 0
    if r0 == 0:
        nc.vector.tensor_copy(out=out_t[:, 0, 0, :], in_=shf[:, 0, :])
    nc.vector.tensor_add(out_t[:, j0:m, 0, :],
                         bu[:, base + j0 - 1:base + m - 1, :],
                         au[:, base + j0:base + m, :])
    nc.vector.tensor_add(out_t[:, 0:m, 1, :], au[:, base:base + m, :],
                         bu[:, base + 1:base + m + 1, :])
    nc.sync.dma_start(
        out=y_d[:, 2 * r0:2 * r1, :],
        in_=out_t[:, 0:m, :, :].rearrange("p a b c -> p (a b) c"))


def _build():
    nc = bacc.Bacc("TRN2", target_bir_lowering=False, debug=False,
                   enable_asserts=True, num_devices=8)

    def din(name, shape, dtp=f32):
        return nc.dram_tensor(name, shape, dtp, kind="ExternalInput").ap()

    d = {
        'x': din("x", [NCH, P, XPR, XPW], f32r),
        'wp': din("wp", [NCH, P, NT, P], f32r),
        'wc': din("wc", [NCH, P, NT, P], f32r),
        'sp': din("sp", [NCH, P, 1]), 'bp': din("bp", [NCH, P, 1]),
        'sc': din("sc", [NCH, P, 1]), 'bc': din("bc", [NCH, P, 1]),
        'wq': din("wq", [NCH, P, CR]), 'wk': din("wk", [NCH, P, CR]),
        'bq': din("bq", [CR, 1]), 'bk': din("bk", [CR, 1]),
        'wv': din("wv", [NCH, P, C], f32r),
        'bv': din("bv", [1, C]),
        'alpha': din("alpha", [1, 1]),
        'wpo': din("wpo", [NCH, P, OC], f32r),
        'bpo': din("bpo", [OC, 1]),
        'wc1': din("wc1", [NCH, P, CR]), 'bc1': din("bc1", [CR, 1]),
        'wc2': din("wc2", [NCH, CR, P]), 'bc2': din("bc2", [NCH, P, 1]),
        'wco': din("wco", [NCH, P, OC]), 'bco': din("bco", [OC, 1]),
        'ident': din("ident", [P, P], fp16),
    }
    y_d = nc.dram_tensor("y", [OC, H, 2 * W], f32, kind="ExternalOutput").ap()

    with tile.TileContext(nc) as tc:
        with tc.tile_pool(name="consts", bufs=2) as pc, \
             tc.tile_pool(name="dram", bufs=1, space="DRAM") as pfd, \
             tc.tile_pool(name="pam_out", bufs=1) as p_pam, \
             tc.tile_pool(name="qk_sb", bufs=1) as pqs:
            pam_sb = p_pam.tile([OC, OWN], f32, name="pam_sb", tag="pam_sb")
            zc_full = p_pam.tile([OC, OWN], f32, name="zc_full", tag="zc_full")
            # consts ride the DVE queue so the x loads own the sync queue
            # from t=0 (PE start gates on x chunk 0 + first conv weights)
            ct = {}
            ct['ident'] = pc.tile([P, P], fp16, name="ident", tag="ident")
            nc.gpsimd.dma_start(out=ct['ident'], in_=d['ident'])
            ct['alpha'] = pc.tile([P, 1], f32, name="alpha_t", tag="alpha_t")
            nc.gpsimd.dma_start(out=ct['alpha'], in_=d['alpha'].to_broadcast([P, 1]))
            for nm, rows in (('bq', CR), ('bk', CR), ('bpo', OC), ('bco', OC),
                             ('bc1', CR)):
                ct[nm] = pc.tile([rows, 1], f32, name=f"{nm}_t", tag=f"{nm}_t")
                nc.gpsimd.dma_start(out=ct[nm], in_=d[nm])
            for nm in ('sp', 'bp', 'sc', 'bc'):
                ct[nm] = [pc.tile([P, 1], f32, name=f"{nm}{i}_t", tag=f"{nm}{i}_t")
                          for i in range(NCH)]
                for i in range(NCH):
                    nc.gpsimd.dma_start(out=ct[nm][i], in_=d[nm][i])
            ct['wpo'] = [pc.tile([P, OC], f32r, name=f"wpo{i}_t", tag=f"wpo{i}_t")
                         for i in range(NCH)]
            for i in range(NCH):
                nc.gpsimd.dma_start(out=ct['wpo'][i], in_=d['wpo'][i])

            kin = pfd.tile([CR, 2, JOWN], fp16, name="kin", tag="kin")
            ct['kout'] = pfd.tile([2, CR, 2, JOWN], fp16, name="kout", tag="kout")
            vin = pfd.tile([P, NJC_OWN, C], fp16, name="vin", tag="vin")
            ct['vout'] = pfd.tile([2, P, NJC_OWN, C], fp16, name="vout", tag="vout")
            min_d = pfd.tile([NCH, P, 1], f32, name="min_d", tag="min_d")
            ct['mout'] = pfd.tile([NCH, P, 1], f32, name="mout", tag="mout")

            # SBUF-resident q/k packs (live into attention)
            ct['qpk'] = pqs.tile([P, OWN], fp16, name="qpk", tag="qpk")
            ct['khd'] = pqs.tile([P, HW], fp16, name="khd", tag="khd")
            ct['klo'] = pqs.tile([CR, HW], fp16, name="klo", tag="klo")

            with tc.tile_pool(name="g_store", bufs=1) as p_g, \
                 tc.tile_pool(name="f16_store", bufs=1) as p_f16:
                g_t = [p_g.tile([P, OWN], fp16, name=f"g{i}", tag=f"g{i}")
                       for i in range(NCH)]
                f16_t = [p_f16.tile([P, OWN], fp16, name=f"h{i}", tag=f"h{i}")
                         for i in range(NCH)]

                with tc.tile_pool(name="q32_pool", bufs=1) as pq32:
                    ct['q32'] = pq32.tile([CR, OWN], f32, name="q32", tag="q32")
                    ct['k32'] = pq32.tile([CR, JOWN], f32, name="k32", tag="k32")
                    ct['qlo'] = pq32.tile([CR, OWN], fp16, name="qlo", tag="qlo")

                    with tc.tile_pool(name="f_store", bufs=1) as p_f, \
                         tc.tile_pool(name="xs", bufs=1) as px:
                        f_t = [p_f.tile([P, OWN], f32r, name=f"f{i}", tag=f"f{i}")
                               for i in range(NCH)]
                        x_t = [px.tile([P, XPR, XPW], f32r, name=f"x{i}",
                                       tag=f"x{i}")
                               for i in range(NCH)]
                        for i in range(NCH):
                            nc.sync.dma_start(out=x_t[i], in_=d['x'][i])

                        def f_store(co, off, ln, z):
                            nc.vector.scalar_tensor_tensor(
                                out=f_t[co][:, off:off + ln], in0=z, scalar=0.2,
                                in1=z, op0=OP.mult, op1=OP.max)

                        def g_store(co, off, ln, z):
                            nc.vector.scalar_tensor_tensor(
                                out=g_t[co][:, off:off + ln], in0=z, scalar=0.2,
                                in1=z, op0=OP.mult, op1=OP.max)

                        _emit_conv(nc, tc, x_t, d['wp'], ct['sp'], ct['bp'],
                                   f_store, None)
                        # prefetch the CAM conv's first weight tile now, in
                        # space (x pool) with no dependency on the gathers
                        cam_w0 = px.tile([P, NT, P], f32r, name="cw0",
                                         tag="cw0")
                        nc.scalar.dma_start(out=cam_w0, in_=d['wc'][0])
                        _emit_qkv(nc, tc, d, ct, f_t, kin, vin)
                        # residual copy of f in fp16 (frees the fp32 f
                        # before attention; q/k/v already consumed fp32)
                        for i in range(NCH):
                            nc.vector.tensor_copy(out=f16_t[i],
                                                  in_=f_t[i].bitcast(f32))
                        _emit_conv(nc, tc, x_t, d['wc'], ct['sc'], ct['bc'],
                                   g_store, cam_w0)
                        _emit_qk_unpack(nc, ct)

                    # CAM partial mean over own token half + pair AllReduce
                    with tc.tile_pool(name="msum_p", bufs=1) as pms:
                        for i in range(NCH):
                            ms = pms.tile([P, 1], f32, name=f"msp{i}",
                                          tag=f"msp{i}")
                            nc.vector.tensor_reduce(
                                out=ms, in_=g_t[i][:, 0:JOWN],
                                axis=mybir.AxisListType.X, op=OP.add)
                            nc.sync.dma_start(out=min_d[i], in_=ms)
                        nc.gpsimd.collective_compute(
                            "AllReduce", mybir.AluOpType.add,
                            replica_groups=GROUPS,
                            ins=[min_d.opt()], outs=[ct['mout'].opt()])

                with tc.tile_pool(name="vt2", bufs=1) as pv2, \
                     tc.tile_pool(name="up", bufs=1) as pu:
                    vt_t = pv2.tile([P, NJC, C], fp16, name="vt2_t", tag="vt2_t")
                    nc.sync.dma_start(out=vt_t[:, 0:NJC_OWN, :], in_=ct['vout'][0])
                    nc.sync.dma_start(out=vt_t[:, NJC_OWN:NJC, :],
                                      in_=ct['vout'][1])
                    n_blocks = (NIC + 1) // 2

                    def upc(k):
                        r0, r1 = 4 * k, 4 * k + 4
                        alo = 0 if k == 0 else 4 * k + 1
                        ahi = OWN_ROWS if k == 7 else 4 * k + 5
                        _emit_up_chunk(nc, pu, pam_sb, zc_full, y_d,
                                       r0, r1, alo, ahi)

                    def up2(k):
                        return lambda: (upc(k), upc(k + 1))

                    _emit_attention(nc, tc, ct, f16_t, pam_sb, vt_t,
                                    list(range(0, 5)))
                    _emit_cam_mlp(nc, tc, d, ct, g_t, zc_full)
                    _emit_attention(nc, tc, ct, f16_t, pam_sb, vt_t,
                                    list(range(5, n_blocks)),
                                    post_cb={5: up2(0), 6: up2(2), 7: up2(4)})
                    upc(6)
                    upc(7)
    nc.compile()
    return nc


_NC_CACHE = None


def _get_nc():
    global _NC_CACHE
    if _NC_CACHE is None:
        _NC_CACHE = _build()
    return _NC_CACHE


_TAP_CI = np.array([t[0] for t in TAPS])
_TAP_DY = np.array([t[1] + 1 for t in TAPS])
_TAP_DX = np.array([t[2] + 1 for t in TAPS])


def _pack_conv(wfull):
    """[C, C, 3, 3] -> [NCH(co), P(ci_local), NT, P(co_local)] lhsT tiles."""
    wr = np.asarray(wfull, np.float32).reshape(NCH, P, NCH, P, 3, 3)
    wt = wr.transpose(0, 2, 4, 5, 3, 1)  # [co, ci, dy, dx, ci_l, co_l]
    taps = wt[:, _TAP_CI, _TAP_DY, _TAP_DX]  # [co, NT, ci_l, co_l]
    return np.ascontiguousarray(taps.transpose(0, 2, 1, 3))


def _packT(w, free):
    """w [free, C] -> [NCH, P, free] lhsT chunks."""
    return np.ascontiguousarray(np.asarray(w, np.float32).T.reshape(NCH, P, free))


def _prep_shared(inputs, flip):
    wp = np.asarray(inputs['W_pam_in'], np.float32)
    wc = np.asarray(inputs['W_cam_in'], np.float32)
    if flip:
        wp = wp[:, :, ::-1, :]
        wc = wc[:, :, ::-1, :]

    def bnfold(g, b, m, v):
        s = (np.asarray(g, np.float32)
             / np.sqrt(np.asarray(v, np.float32) + EPS)).astype(np.float32)
        bb = (np.asarray(b, np.float32)
              - np.asarray(m, np.float32) * s).astype(np.float32)
        return s.reshape(NCH, P, 1), bb.reshape(NCH, P, 1)

    sp, bp = bnfold(inputs['pam_gamma'], inputs['pam_beta'],
                    inputs['pam_mean'], inputs['pam_var'])
    sc, bc = bnfold(inputs['cam_gamma'], inputs['cam_beta'],
                    inputs['cam_mean'], inputs['cam_var'])
    # Wc2 [C, CR] -> lhsT chunks [NCH, CR, P]
    wc2 = np.ascontiguousarray(
        np.asarray(inputs['Wc2'], np.float32).reshape(NCH, P, CR).transpose(0, 2, 1))
    return {
        'wp': _pack_conv(wp), 'wc': _pack_conv(wc),
        'sp': sp, 'bp': bp, 'sc': sc, 'bc': bc,
        'wq': _packT(inputs['Wq'], CR), 'wk': _packT(inputs['Wk'], CR),
        'bq': np.asarray(inputs['bq'], np.float32).reshape(CR, 1),
        'bk': np.asarray(inputs['bk'], np.float32).reshape(CR, 1),
        'wv': _packT(inputs['Wv'], C),
        'bv': np.asarray(inputs['bv'], np.float32).reshape(1, C),
        'alpha': np.asarray(inputs['alpha'], np.float32).reshape(1, 1),
        'wpo': _packT(inputs['W_pam_out'], OC),
        'bpo': np.asarray(inputs['b_pam_out'], np.float32).reshape(OC, 1),
        'wc1': _packT(np.asarray(inputs['Wc1'], np.float32) / HW, CR),
        'bc1': np.asarray(inputs['bc1'], np.float32).reshape(CR, 1),
        'wc2': wc2,
        'bc2': np.asarray(inputs['bc2'], np.float32).reshape(NCH, P, 1),
        'wco': _packT(inputs['W_cam_out'], OC),
        'ident': np.eye(P, dtype=np.float16),
        'bco': np.asarray(inputs['b_cam_out'], np.float32).reshape(OC, 1),
    }


def _make_in_maps(inputs):
    x = np.asarray(inputs['x'], np.float32)  # [4, 512, 64, 64]
    shared = {f: _prep_shared(inputs, f) for f in (False, True)}
    in_maps = []
    for c in range(8):
        s, flip = c // 2, c % 2
        xs = x[s]
        if flip:
            xs = xs[:, ::-1, :]
        xp = np.zeros((C, XPR, XPW), np.float32)
        xp[:, 1:XPR, 1:W + 1] = xs[:, 0:XR, :]
        m = dict(shared[bool(flip)])
        m['x'] = np.ascontiguousarray(xp.reshape(NCH, P, XPR, XPW))
        in_maps.append(m)
    return in_maps


def kernel(**inputs):
    nc = _get_nc()
    in_maps = _make_in_maps(inputs)
    res = run_bass_kernel_spmd(nc, in_maps, list(range(8)))
    out = np.empty((4, OC, 2 * H, 2 * W), np.float32)
    for c in range(8):
        s, flip = c // 2, c % 2
        o = res.results[c]['y']  # [64, 64, 128]
        if flip:
            out[s, :, H:2 * H, :] = o[:, ::-1, :]
        else:
            out[s, :, 0:H, :] = o
    return out
